# revision 2
# baseline (speedup 1.0000x reference)
"""BattleTransformer forward for Trainium2 — 8-core batch-data-parallel.

Contract: kernel(**inputs) takes the FULL (unsharded) inputs exactly as
produced by setup_inputs() and returns the FULL output (same structure the
reference returns).  Inside, the batch (B=1024) is sharded 8 ways (128 per
NeuronCore), params are replicated, and the whole forward (hex scatter,
4-layer transformer, heads) executes on the 8 trn2 NeuronCores as one SPMD
program.  Host code only splits/stacks the batch dimension and gathers the
outputs.
"""

import numpy as np
import jax
import jax.numpy as jnp

H = 187          # BATTLEFIELD_HEXES
MS = 20          # MAX_STACKS
MOB = 10         # MAX_OBSTACLES
D = 128          # d_model
NH = 4           # n_heads
NL = 4           # n_layers
CE = 16          # creature_embed_dim
NCT = 256        # NUM_CREATURE_TYPES
HEXC = 29        # HEX_CONTINUOUS_DIM
B = 1024         # full batch
NCORES = 8

_BATCH_KEYS = ("scalars", "stacks", "obstacles", "reachable_hexes", "n_stacks")


def _ln(x, g, b):
    m = x.mean(-1, keepdims=True)
    v = jnp.mean((x - m) ** 2, -1, keepdims=True)
    return (x - m) / jnp.sqrt(v + 1e-5) * g + b


def _mlp(x, w1, b1, w2, b2):
    return jax.nn.relu(x @ w1.T + b1) @ w2.T + b2


def _forward(p, n_stacks):
    scalars = p['scalars']
    s = p['stacks']
    Bl = s.shape[0]
    pos = s[:, :, 18].astype(jnp.int32)                       # POSITION
    alive = s[:, :, 23]                                       # ALIVE
    idx_mask = jnp.arange(MS)[None, :] < n_stacks             # (Bl, MS)
    valid = (pos >= 0) & (pos < H) & (alive >= 0.5) & idx_mask
    max_hp = jnp.maximum(s[:, :, 4], 1.0)
    is_active = (s[:, :, 0] == scalars[:, 2:3]).astype(jnp.float32)
    is_ally = (s[:, :, 20] == scalars[:, 3:4]).astype(jnp.float32)
    zeros = jnp.zeros_like(alive)
    feat = jnp.stack([
        jnp.ones_like(alive), s[:, :, 2] / 1000.0, s[:, :, 3] / max_hp,
        s[:, :, 8] / 100.0, s[:, :, 9] / 100.0, s[:, :, 10] / 100.0, s[:, :, 11] / 100.0,
        s[:, :, 12] / 100.0, s[:, :, 13] / 100.0, s[:, :, 14] / 100.0, s[:, :, 15] / 100.0,
        s[:, :, 16] / 20.0, s[:, :, 17] / 20.0, s[:, :, 20], is_ally, alive,
        s[:, :, 24], s[:, :, 25], s[:, :, 26], s[:, :, 27], s[:, :, 28], s[:, :, 29],
        s[:, :, 30], s[:, :, 31] / 30.0, s[:, :, 33] / 5.0, s[:, :, 34] / 10.0,
        is_active, zeros, zeros], axis=-1)                    # (Bl, MS, 29)
    # --- scatter-free (one-hot matmul) formulation of the hex scatter ---
    # positions are unique per batch row, so scatter == one-hot contraction.
    safe_pos = jnp.where(valid, pos, H)                       # dummy slot H for invalid
    hexes = jnp.arange(H, dtype=jnp.int32)
    onehot = (safe_pos[:, :, None] == hexes[None, None, :]).astype(jnp.float32)
    feat_v = jnp.where(valid[..., None], feat, 0.0)           # (Bl,MS,29)
    hex_cont = jnp.einsum('bsh,bsc->bhc', onehot, feat_v)     # (Bl,H,29)
    # creature embedding: gather-free via one-hot over the 256-entry table
    cid = jnp.minimum(s[:, :, 1].astype(jnp.int32), NCT - 1)
    cid_oh = (cid[:, :, None] == jnp.arange(NCT, dtype=jnp.int32)[None, None, :])
    emb_stack = cid_oh.astype(jnp.float32) @ p['creature_emb']  # (Bl,MS,CE)
    emb_stack = jnp.where(valid[..., None], emb_stack, 0.0)
    emb_grid = jnp.einsum('bsh,bse->bhe', onehot, emb_stack)    # (Bl,H,CE)
    # reachable + obstacle channels
    reach = p['reachable_hexes']
    opos = p['obstacles'][:, :, 2].astype(jnp.int32)
    ovalid = (p['obstacles'][:, :, 0] > 0) & (opos >= 0) & (opos < H)
    obs_oh = (jnp.where(ovalid, opos, H)[:, :, None] == hexes[None, None, :])
    obs_ch = jnp.max(obs_oh & ovalid[:, :, None], axis=1).astype(jnp.float32)
    hex_cont = jnp.concatenate(
        [hex_cont[:, :, :27], reach[..., None], obs_ch[..., None]], axis=-1)
    hex_feat = jnp.concatenate([hex_cont, emb_grid], -1)      # (Bl,H,45)

    hex_tok = _ln(hex_feat @ p['hex_proj_w'].T + p['hex_proj_b'],
                  p['hex_ln_g'], p['hex_ln_b'])
    hex_tok = hex_tok + p['hex_pos_emb'][None] + p['tok_type_emb'][0]

    sc = scalars
    atk = jnp.stack([sc[:, 8], sc[:, 11] / 300.0, sc[:, 10] / 10.0, sc[:, 12] / 10.0,
                     jnp.zeros(Bl)], -1)
    dfn = jnp.stack([sc[:, 14], sc[:, 17] / 300.0, sc[:, 16] / 10.0, sc[:, 18] / 10.0,
                     jnp.ones(Bl)], -1)
    atk_t = _ln(atk @ p['hero_proj_w'].T + p['hero_proj_b'],
                p['hero_ln_g'], p['hero_ln_b']) + p['tok_type_emb'][1]
    def_t = _ln(dfn @ p['hero_proj_w'].T + p['hero_proj_b'],
                p['hero_ln_g'], p['hero_ln_b']) + p['tok_type_emb'][2]
    tokens = jnp.concatenate([hex_tok, atk_t[:, None], def_t[:, None]], 1)  # (Bl,189,D)

    gfeat = jnp.stack([sc[:, 1] / 50.0, sc[:, 4] / 10.0, sc[:, 5] / 10.0,
                       sc[:, 6], sc[:, 3]], -1)
    tokens = tokens + (gfeat @ p['global_w'].T + p['global_b'])[:, None]

    S = tokens.shape[1]
    hd = D // NH
    for l in range(NL):  # pre-LN transformer encoder (eval mode)
        x = _ln(tokens, p['tf_ln1_g'][l], p['tf_ln1_b'][l])
        qkv = x @ p['tf_qkv_w'][l].T + p['tf_qkv_b'][l]
        q, k, v = jnp.split(qkv, 3, -1)
        rs = lambda t: t.reshape(Bl, S, NH, hd).transpose(0, 2, 1, 3)
        q, k, v = rs(q), rs(k), rs(v)
        att = jax.nn.softmax(jnp.einsum('bhqd,bhkd->bhqk', q, k) / jnp.sqrt(float(hd)), -1)
        o = jnp.einsum('bhqk,bhkd->bhqd', att, v).transpose(0, 2, 1, 3).reshape(Bl, S, D)
        tokens = tokens + o @ p['tf_o_w'][l].T + p['tf_o_b'][l]
        y = _ln(tokens, p['tf_ln2_g'][l], p['tf_ln2_b'][l])
        tokens = tokens + jax.nn.relu(y @ p['tf_ff1_w'][l].T + p['tf_ff1_b'][l]) \
            @ p['tf_ff2_w'][l].T + p['tf_ff2_b'][l]
    tokens = _ln(tokens, p['post_ln_g'], p['post_ln_b'])

    hex_out = tokens[:, :H]
    pooled = tokens.mean(1)
    action_type_logits = _mlp(pooled, p['at_w1'], p['at_b1'], p['at_w2'], p['at_b2'])
    hex_logits = _mlp(hex_out, p['hex_w1'], p['hex_b1'], p['hex_w2'], p['hex_b2']).squeeze(-1)
    cpos = jnp.clip(pos, 0, H - 1)
    pos_oh = (cpos[:, :, None] == jnp.arange(H, dtype=jnp.int32)[None, None, :])
    tgt_tok = jnp.einsum('bsh,bhd->bsd', pos_oh.astype(jnp.float32), hex_out)
    tgt = _mlp(tgt_tok, p['tg_w1'], p['tg_b1'], p['tg_w2'], p['tg_b2']).squeeze(-1)
    target_logits = jnp.where(valid, tgt, -1e9)
    value = _mlp(pooled, p['val_w1'], p['val_b1'], p['val_w2'], p['val_b2']).squeeze(-1)
    return (action_type_logits, hex_logits, target_logits, value)


def _shard_forward(batch, params):
    p = dict(params)
    p.update({k: batch[k] for k in _BATCH_KEYS if k != 'n_stacks'})
    return _forward(p, batch['n_stacks'])


_PMAP_CACHE = {}


def _get_pmapped(ndev):
    if ndev not in _PMAP_CACHE:
        _PMAP_CACHE[ndev] = jax.pmap(
            _shard_forward, in_axes=(0, None), devices=jax.devices()[:ndev])
    return _PMAP_CACHE[ndev]


def kernel(**inputs) -> np.ndarray:
    params = {k: np.asarray(v) for k, v in inputs.items() if k not in _BATCH_KEYS}
    ndev = min(NCORES, jax.local_device_count())
    bs = inputs['scalars'].shape[0]
    assert bs % ndev == 0
    shard = bs // ndev
    batch = {
        k: np.asarray(inputs[k]).reshape((ndev, shard) + np.asarray(inputs[k]).shape[1:])
        for k in _BATCH_KEYS
    }
    fn = _get_pmapped(ndev)
    outs = fn(batch, params)
    outs = jax.tree_util.tree_map(
        lambda a: np.asarray(a).reshape((bs,) + a.shape[2:]), outs)
    return outs


# revision 3
# speedup vs baseline: 1.0146x; 1.0146x over previous
"""BattleTransformer forward for Trainium2 — 8-core batch-data-parallel.

Contract: kernel(**inputs) takes the FULL (unsharded) inputs exactly as
produced by setup_inputs() and returns the FULL output (same structure the
reference returns).  Inside, the batch (B=1024) is sharded 8 ways (128 per
NeuronCore), params are replicated, and the whole forward (hex scatter,
4-layer transformer, heads) executes on the 8 trn2 NeuronCores as one SPMD
program.  Host code only splits/stacks the batch dimension and gathers the
outputs.
"""

import numpy as np
import jax
import jax.numpy as jnp

H = 187          # BATTLEFIELD_HEXES
MS = 20          # MAX_STACKS
MOB = 10         # MAX_OBSTACLES
D = 128          # d_model
NH = 4           # n_heads
NL = 4           # n_layers
CE = 16          # creature_embed_dim
NCT = 256        # NUM_CREATURE_TYPES
HEXC = 29        # HEX_CONTINUOUS_DIM
B = 1024         # full batch
NCORES = 8

_BATCH_KEYS = ("scalars", "stacks", "obstacles", "reachable_hexes", "n_stacks")


def _ln(x, g, b):
    m = x.mean(-1, keepdims=True)
    v = jnp.mean((x - m) ** 2, -1, keepdims=True)
    return (x - m) / jnp.sqrt(v + 1e-5) * g + b


def _mlp(x, w1, b1, w2, b2):
    return jax.nn.relu(x @ w1.T + b1) @ w2.T + b2


_BF = jnp.bfloat16


def _mmT(x, w):
    """x @ w.T with bf16 operands, fp32 accumulation (PE runs bf16 at 2x)."""
    return jax.lax.dot_general(
        x.astype(_BF), w.astype(_BF),
        (((x.ndim - 1,), (1,)), ((), ())),
        preferred_element_type=jnp.float32)


def _forward(p, n_stacks):
    scalars = p['scalars']
    s = p['stacks']
    Bl = s.shape[0]
    pos = s[:, :, 18].astype(jnp.int32)                       # POSITION
    alive = s[:, :, 23]                                       # ALIVE
    idx_mask = jnp.arange(MS)[None, :] < n_stacks             # (Bl, MS)
    valid = (pos >= 0) & (pos < H) & (alive >= 0.5) & idx_mask
    max_hp = jnp.maximum(s[:, :, 4], 1.0)
    is_active = (s[:, :, 0] == scalars[:, 2:3]).astype(jnp.float32)
    is_ally = (s[:, :, 20] == scalars[:, 3:4]).astype(jnp.float32)
    zeros = jnp.zeros_like(alive)
    feat = jnp.stack([
        jnp.ones_like(alive), s[:, :, 2] / 1000.0, s[:, :, 3] / max_hp,
        s[:, :, 8] / 100.0, s[:, :, 9] / 100.0, s[:, :, 10] / 100.0, s[:, :, 11] / 100.0,
        s[:, :, 12] / 100.0, s[:, :, 13] / 100.0, s[:, :, 14] / 100.0, s[:, :, 15] / 100.0,
        s[:, :, 16] / 20.0, s[:, :, 17] / 20.0, s[:, :, 20], is_ally, alive,
        s[:, :, 24], s[:, :, 25], s[:, :, 26], s[:, :, 27], s[:, :, 28], s[:, :, 29],
        s[:, :, 30], s[:, :, 31] / 30.0, s[:, :, 33] / 5.0, s[:, :, 34] / 10.0,
        is_active, zeros, zeros], axis=-1)                    # (Bl, MS, 29)
    # --- scatter-free (one-hot matmul) formulation of the hex scatter ---
    # positions are unique per batch row, so scatter == one-hot contraction.
    safe_pos = jnp.where(valid, pos, H)                       # dummy slot H for invalid
    hexes = jnp.arange(H, dtype=jnp.int32)
    onehot = (safe_pos[:, :, None] == hexes[None, None, :]).astype(jnp.float32)
    feat_v = jnp.where(valid[..., None], feat, 0.0)           # (Bl,MS,29)
    hex_cont = jnp.einsum('bsh,bsc->bhc', onehot, feat_v)     # (Bl,H,29)
    # creature embedding: gather-free via one-hot over the 256-entry table
    cid = jnp.minimum(s[:, :, 1].astype(jnp.int32), NCT - 1)
    cid_oh = (cid[:, :, None] == jnp.arange(NCT, dtype=jnp.int32)[None, None, :])
    emb_stack = cid_oh.astype(jnp.float32) @ p['creature_emb']  # (Bl,MS,CE)
    emb_stack = jnp.where(valid[..., None], emb_stack, 0.0)
    emb_grid = jnp.einsum('bsh,bse->bhe', onehot, emb_stack)    # (Bl,H,CE)
    # reachable + obstacle channels
    reach = p['reachable_hexes']
    opos = p['obstacles'][:, :, 2].astype(jnp.int32)
    ovalid = (p['obstacles'][:, :, 0] > 0) & (opos >= 0) & (opos < H)
    obs_oh = (jnp.where(ovalid, opos, H)[:, :, None] == hexes[None, None, :])
    obs_ch = jnp.max(obs_oh & ovalid[:, :, None], axis=1).astype(jnp.float32)
    hex_cont = jnp.concatenate(
        [hex_cont[:, :, :27], reach[..., None], obs_ch[..., None]], axis=-1)
    hex_feat = jnp.concatenate([hex_cont, emb_grid], -1)      # (Bl,H,45)

    hex_tok = _ln(hex_feat @ p['hex_proj_w'].T + p['hex_proj_b'],
                  p['hex_ln_g'], p['hex_ln_b'])
    hex_tok = hex_tok + p['hex_pos_emb'][None] + p['tok_type_emb'][0]

    sc = scalars
    atk = jnp.stack([sc[:, 8], sc[:, 11] / 300.0, sc[:, 10] / 10.0, sc[:, 12] / 10.0,
                     jnp.zeros(Bl)], -1)
    dfn = jnp.stack([sc[:, 14], sc[:, 17] / 300.0, sc[:, 16] / 10.0, sc[:, 18] / 10.0,
                     jnp.ones(Bl)], -1)
    atk_t = _ln(atk @ p['hero_proj_w'].T + p['hero_proj_b'],
                p['hero_ln_g'], p['hero_ln_b']) + p['tok_type_emb'][1]
    def_t = _ln(dfn @ p['hero_proj_w'].T + p['hero_proj_b'],
                p['hero_ln_g'], p['hero_ln_b']) + p['tok_type_emb'][2]
    tokens = jnp.concatenate([hex_tok, atk_t[:, None], def_t[:, None]], 1)  # (Bl,189,D)

    gfeat = jnp.stack([sc[:, 1] / 50.0, sc[:, 4] / 10.0, sc[:, 5] / 10.0,
                       sc[:, 6], sc[:, 3]], -1)
    tokens = tokens + (gfeat @ p['global_w'].T + p['global_b'])[:, None]

    S = tokens.shape[1]
    hd = D // NH
    for l in range(NL):  # pre-LN transformer encoder (eval mode)
        x = _ln(tokens, p['tf_ln1_g'][l], p['tf_ln1_b'][l])
        qkv = _mmT(x, p['tf_qkv_w'][l]) + p['tf_qkv_b'][l]
        q, k, v = jnp.split(qkv, 3, -1)
        rs = lambda t: t.reshape(Bl, S, NH, hd).transpose(0, 2, 1, 3)
        q, k, v = rs(q), rs(k), rs(v)
        att = jax.nn.softmax(
            jnp.einsum('bhqd,bhkd->bhqk', q.astype(_BF), k.astype(_BF),
                       preferred_element_type=jnp.float32) / jnp.sqrt(float(hd)), -1)
        o = jnp.einsum('bhqk,bhkd->bhqd', att.astype(_BF), v.astype(_BF),
                       preferred_element_type=jnp.float32)
        o = o.transpose(0, 2, 1, 3).reshape(Bl, S, D)
        tokens = tokens + _mmT(o, p['tf_o_w'][l]) + p['tf_o_b'][l]
        y = _ln(tokens, p['tf_ln2_g'][l], p['tf_ln2_b'][l])
        tokens = tokens + _mmT(
            jax.nn.relu(_mmT(y, p['tf_ff1_w'][l]) + p['tf_ff1_b'][l]),
            p['tf_ff2_w'][l]) + p['tf_ff2_b'][l]
    tokens = _ln(tokens, p['post_ln_g'], p['post_ln_b'])

    hex_out = tokens[:, :H]
    pooled = tokens.mean(1)
    action_type_logits = _mlp(pooled, p['at_w1'], p['at_b1'], p['at_w2'], p['at_b2'])
    hex_logits = _mlp(hex_out, p['hex_w1'], p['hex_b1'], p['hex_w2'], p['hex_b2']).squeeze(-1)
    cpos = jnp.clip(pos, 0, H - 1)
    pos_oh = (cpos[:, :, None] == jnp.arange(H, dtype=jnp.int32)[None, None, :])
    tgt_tok = jnp.einsum('bsh,bhd->bsd', pos_oh.astype(jnp.float32), hex_out)
    tgt = _mlp(tgt_tok, p['tg_w1'], p['tg_b1'], p['tg_w2'], p['tg_b2']).squeeze(-1)
    target_logits = jnp.where(valid, tgt, -1e9)
    value = _mlp(pooled, p['val_w1'], p['val_b1'], p['val_w2'], p['val_b2']).squeeze(-1)
    return (action_type_logits, hex_logits, target_logits, value)


def _shard_forward(batch, params):
    p = dict(params)
    p.update({k: batch[k] for k in _BATCH_KEYS if k != 'n_stacks'})
    return _forward(p, batch['n_stacks'])


_PMAP_CACHE = {}


def _get_pmapped(ndev):
    if ndev not in _PMAP_CACHE:
        _PMAP_CACHE[ndev] = jax.pmap(
            _shard_forward, in_axes=(0, None), devices=jax.devices()[:ndev])
    return _PMAP_CACHE[ndev]


def kernel(**inputs) -> np.ndarray:
    params = {k: np.asarray(v) for k, v in inputs.items() if k not in _BATCH_KEYS}
    ndev = min(NCORES, jax.local_device_count())
    bs = inputs['scalars'].shape[0]
    assert bs % ndev == 0
    shard = bs // ndev
    batch = {
        k: np.asarray(inputs[k]).reshape((ndev, shard) + np.asarray(inputs[k]).shape[1:])
        for k in _BATCH_KEYS
    }
    fn = _get_pmapped(ndev)
    outs = fn(batch, params)
    outs = jax.tree_util.tree_map(
        lambda a: np.asarray(a).reshape((bs,) + a.shape[2:]), outs)
    return outs
